# revision 2
# baseline (speedup 1.0000x reference)
"""Trainium2 Bass kernel: AnaphoricityScorer (wl-coref pair FFNN scorer).

Data-parallel over the 512-row mention batch across 8 NeuronCores (64 rows
per core).  Per core (3200 pairs):

  1. The gather b = all_mentions[top_indices] is done with the GPSIMD
     dma_gather(transpose=True) custom DMA, which lands the gathered fp16
     embeddings TRANSPOSED in SBUF: out[p, c, n] = table[idx_n, 128c+p].
     That puts the contraction dim (embedding) on partitions, which is
     exactly the layout the TensorEngine needs for the moving operand.
  2. s = a * b (similarity) is one DVE multiply against a pre-broadcast
     a^T tile (host-side replication of the mention embeddings).
  3. hT[h, pair] = W1b^T b + W1s^T s + W1p^T pw + (a@W1a broadcast) is
     accumulated in PSUM via fp16 matmuls with the W1 chunks as stationary
     operands (the a-term enters via a one-hot moving operand against the
     on-device-computed ma = mentions@W1a).
  4. One ScalarEngine activation applies  leaky_relu(hT + b1)  straight out
     of PSUM into SBUF (fp16).
  5. Layer 2 uses hrelu slices as the stationary operand: out[pair, 1] =
     hrelu_slice^T @ W_out; + b_out (activation bias) + rough (DVE add).

Pair order is "antecedent-major" (p' = j*64 + i) so the a-broadcast is a
clean 64-wide repeat; all permutation/layout work is host-side.
"""

import numpy as np

N_MENTIONS = 10000
BATCH = 512
N_ANTS = 50
EMB = 1024
PW = 64
HID = 128
N_CORES = 8
R = BATCH // N_CORES            # 64 rows per core
NPAIR = R * N_ANTS              # 3200 pairs per core
BLK = 640                       # pairs per pipeline block
NB = NPAIR // BLK               # 5 blocks
NG = BLK // 128                 # 5 layer-2 column groups per block
NCH = EMB // 128                # 8 embedding chunks
EPS = 1e-7
LEAKY = 0.01

_CACHE = {}


def _build():
    """Build + compile the (SPMD, per-core identical) Bass program."""
    if "nc" in _CACHE:
        return _CACHE["nc"]
    from concourse import bacc, mybir
    import concourse.tile as tile

    f16, f32, i16 = mybir.dt.float16, mybir.dt.float32, mybir.dt.int16
    AF = mybir.ActivationFunctionType
    nc = bacc.Bacc()

    def inp(name, shape, dtype):
        return nc.declare_dram_parameter(name, list(shape), dtype, isOutput=False)

    table = inp("table", [N_MENTIONS, EMB], f16)
    idx = inp("idx", [128, NPAIR // 16], i16)
    wb = inp("wb", [128, NCH * HID], f16)
    ws = inp("ws", [128, NCH * HID], f16)
    wa = inp("wa", [128, NCH * HID], f16)
    mT = inp("mT", [128, NCH * R], f16)
    aT = inp("aT", [128, NCH * BLK], f16)
    w1p = inp("w1p", [PW, HID], f16)
    pwT = inp("pwT", [PW, NPAIR], f16)
    e64 = inp("e64", [R, NPAIR], f16)
    wout = inp("wout", [HID, 1], f16)
    b1c = inp("b1c", [HID, 1], f32)
    boutc = inp("boutc", [128, 1], f32)
    rough = inp("rough", [128, NPAIR // 128], f32)
    out = nc.declare_dram_parameter("out", [128, NPAIR // 128], f32, isOutput=True)

    with tile.TileContext(nc) as tc:
        with (
            tc.tile_pool(name="const", bufs=1) as cp,
            tc.tile_pool(name="bt", bufs=2) as btp,
            tc.tile_pool(name="st", bufs=2) as stp,
            tc.tile_pool(name="hr", bufs=2) as hrp,
            tc.tile_pool(name="sm", bufs=2) as smp,
            tc.tile_pool(name="psH", bufs=2, space="PSUM") as psH,
            tc.tile_pool(name="psS", bufs=2, space="PSUM") as psS,
            tc.tile_pool(name="psM", bufs=1, space="PSUM") as psM,
        ):
            def load(param, shape, dtype, tag):
                t = cp.tile(shape, dtype, tag=tag)
                nc.sync.dma_start(out=t[:], in_=param[:])
                return t

            idx_sb = load(idx, [128, NPAIR // 16], i16, "idx")
            wb_sb = load(wb, [128, NCH * HID], f16, "wb")
            ws_sb = load(ws, [128, NCH * HID], f16, "ws")
            wa_sb = load(wa, [128, NCH * HID], f16, "wa")
            mT_sb = load(mT, [128, NCH * R], f16, "mT")
            aT_sb = load(aT, [128, NCH * BLK], f16, "aT")
            w1p_sb = load(w1p, [PW, HID], f16, "w1p")
            pwT_sb = load(pwT, [PW, NPAIR], f16, "pwT")
            e64_sb = load(e64, [R, NPAIR], f16, "e64")
            wout_sb = load(wout, [HID, 1], f16, "wout")
            b1_sb = load(b1c, [HID, 1], f32, "b1c")
            bout_sb = load(boutc, [128, 1], f32, "boutc")
            rough_sb = load(rough, [128, NPAIR // 128], f32, "rough")

            scores_sb = cp.tile([128, NPAIR // 128], f32, tag="scores")

            # ma = mentions_shard @ W1a  -> [R, HID]
            ma_ps = psM.tile([R, HID], f32)
            for c in range(NCH):
                nc.tensor.matmul(
                    ma_ps[:],
                    lhsT=mT_sb[:, c * R:(c + 1) * R],
                    rhs=wa_sb[:, c * HID:(c + 1) * HID],
                    start=(c == 0),
                    stop=(c == NCH - 1),
                )
            ma_sb = cp.tile([R, HID], f16)
            nc.scalar.activation(ma_sb[:], ma_ps[:], AF.Copy)

            nsub = [(0, 512), (512, BLK)]
            for b in range(NB):
                bt = btp.tile([128, NCH * BLK], f16)
                nc.gpsimd.dma_gather(
                    out_ap=bt[:].rearrange("p (c n) -> p c n", c=NCH),
                    in_ap=table[:],
                    idxs_ap=idx_sb[:, b * (BLK // 16):(b + 1) * (BLK // 16)],
                    num_idxs=BLK,
                    num_idxs_reg=BLK,
                    elem_size=EMB,
                    transpose=True,
                )
                st = stp.tile([128, NCH * BLK], f16)
                nc.vector.tensor_mul(st[:], bt[:], aT_sb[:])

                hT = psH.tile([128, BLK], f32)
                for lo, hi in nsub:
                    for c in range(NCH):
                        nc.tensor.matmul(
                            hT[:, lo:hi],
                            lhsT=wb_sb[:, c * HID:(c + 1) * HID],
                            rhs=bt[:, c * BLK + lo:c * BLK + hi],
                            start=(c == 0),
                            stop=False,
                        )
                    for c in range(NCH):
                        nc.tensor.matmul(
                            hT[:, lo:hi],
                            lhsT=ws_sb[:, c * HID:(c + 1) * HID],
                            rhs=st[:, c * BLK + lo:c * BLK + hi],
                            start=False,
                            stop=False,
                        )
                    nc.tensor.matmul(
                        hT[:, lo:hi],
                        lhsT=w1p_sb[:],
                        rhs=pwT_sb[:, b * BLK + lo:b * BLK + hi],
                        start=False,
                        stop=False,
                    )
                    nc.tensor.matmul(
                        hT[:, lo:hi],
                        lhsT=ma_sb[:],
                        rhs=e64_sb[:, b * BLK + lo:b * BLK + hi],
                        start=False,
                        stop=True,
                    )

                hr = hrp.tile([128, BLK], f16)
                nc.scalar.activation(
                    hr[:], hT[:], AF.Lrelu, bias=b1_sb[:], scale=1.0, alpha=LEAKY
                )

                sc = psS.tile([128, NG], f32)
                for g in range(NG):
                    nc.tensor.matmul(
                        sc[:, g:g + 1],
                        lhsT=hr[:, g * 128:(g + 1) * 128],
                        rhs=wout_sb[:],
                        start=True,
                        stop=True,
                    )
                tmp = smp.tile([128, NG], f32)
                nc.scalar.activation(
                    tmp[:], sc[:], AF.Identity, bias=bout_sb[:], scale=1.0
                )
                nc.vector.tensor_add(
                    scores_sb[:, b * NG:(b + 1) * NG],
                    tmp[:],
                    rough_sb[:, b * NG:(b + 1) * NG],
                )

            nc.sync.dma_start(out=out[:], in_=scores_sb[:])

    nc.compile()
    _CACHE["nc"] = nc
    return nc


def _chunkT(w):
    # [1024, 128] -> [128, 8*128] fp16: column c*128+h holds W[c*128+r, h]
    return np.ascontiguousarray(
        w.reshape(NCH, 128, HID).transpose(1, 0, 2).reshape(128, NCH * HID)
    ).astype(np.float16)


def _host_shared(inputs):
    table = np.asarray(inputs["all_mentions"], np.float32).astype(np.float16)
    W1 = np.asarray(inputs["W1"], np.float32)
    w1a, w1b, w1s, w1p = W1[:1024], W1[1024:2048], W1[2048:3072], W1[3072:]
    shared = {
        "table": np.ascontiguousarray(table),
        "wb": _chunkT(w1b),
        "ws": _chunkT(w1s),
        "wa": _chunkT(w1a),
        "w1p": np.ascontiguousarray(w1p).astype(np.float16),
        "e64": np.ascontiguousarray(np.tile(np.eye(R, dtype=np.float16), (1, N_ANTS))),
        "wout": np.asarray(inputs["W_out"], np.float32).astype(np.float16),
        "b1c": np.asarray(inputs["b1"], np.float32).reshape(HID, 1).copy(),
        "boutc": np.full((128, 1), np.asarray(inputs["b_out"], np.float32).reshape(())),
    }
    return shared


def _host_core(inputs, c):
    sl = slice(c * R, (c + 1) * R)
    m = np.asarray(inputs["mentions_batch"], np.float32)[sl]          # [64, 1024]
    pw = np.asarray(inputs["pw_batch"], np.float32)[sl]               # [64, 50, 64]
    idx = np.asarray(inputs["top_indices_batch"])[sl].astype(np.int64)
    rough = np.asarray(inputs["top_rough_scores_batch"], np.float32)[sl]

    idx_perm = idx.T.reshape(NPAIR).astype(np.int16)                  # p' = j*R + i
    idx16 = np.concatenate(
        [
            np.tile(
                idx_perm[b * BLK:(b + 1) * BLK].reshape(BLK // 16, 16).T, (8, 1)
            )
            for b in range(NB)
        ],
        axis=1,
    )                                                                 # [128, 200]

    mT = m.reshape(R, NCH, 128).transpose(2, 1, 0).reshape(128, NCH * R)
    aT = np.broadcast_to(
        mT.reshape(128, NCH, 1, R), (128, NCH, BLK // R, R)
    ).reshape(128, NCH * BLK)
    pwT = pw.transpose(1, 0, 2).reshape(NPAIR, PW).T                  # [64, 3200]
    rough_pp = rough.T.reshape(NPAIR).reshape(NPAIR // 128, 128).T    # [128, 25]

    return {
        "idx": np.ascontiguousarray(idx16),
        "mT": np.ascontiguousarray(mT).astype(np.float16),
        "aT": np.ascontiguousarray(aT).astype(np.float16),
        "pwT": np.ascontiguousarray(pwT).astype(np.float16),
        "rough": np.ascontiguousarray(rough_pp).astype(np.float32),
    }


def make_in_maps(inputs):
    shared = _host_shared(inputs)
    return [{**shared, **_host_core(inputs, c)} for c in range(N_CORES)]


def assemble_output(inputs, results):
    """results: list of per-core dicts with 'out' [128, 25] -> [512, 51] f32."""
    rough = np.asarray(inputs["top_rough_scores_batch"], np.float32)
    scores = np.empty((BATCH, N_ANTS), np.float32)
    for c in range(N_CORES):
        out_flat = np.asarray(results[c]["out"], np.float32).T.reshape(NPAIR)
        scores[c * R:(c + 1) * R] = out_flat.reshape(N_ANTS, R).T
    del rough
    dummy = np.full((BATCH, 1), EPS, np.float32)
    return np.concatenate([dummy, scores], axis=1)


def run(inputs, trace=False, **kwargs):
    """Compile (cached), run on 8 cores, return (output, BassKernelResults)."""
    from concourse.bass_utils import run_bass_kernel_spmd

    nc = _build()
    in_maps = make_in_maps(inputs)
    res = run_bass_kernel_spmd(
        nc, in_maps, core_ids=list(range(N_CORES)), trace=trace, **kwargs
    )
    return assemble_output(inputs, res.results), res


def kernel(**inputs) -> np.ndarray:
    out, _ = run(inputs, trace=False)
    return out


# revision 6
# speedup vs baseline: 1.1658x; 1.1658x over previous
"""Trainium2 Bass kernel: AnaphoricityScorer (wl-coref pair FFNN scorer).

Data-parallel over the 512-row mention batch across 8 NeuronCores (64 rows
per core).  Per core (3200 pairs):

  1. The gather b = all_mentions[top_indices] is done with the GPSIMD
     dma_gather(transpose=True) custom DMA, which lands the gathered fp16
     embeddings TRANSPOSED in SBUF: out[p, c, n] = table[idx_n, 128c+p].
     That puts the contraction dim (embedding) on partitions, which is
     exactly the layout the TensorEngine needs for the moving operand.
  2. s = a * b (similarity) is one DVE multiply against a pre-broadcast
     a^T tile (host-side replication of the mention embeddings).
  3. hT[h, pair] = W1b^T b + W1s^T s + W1p^T pw + (a@W1a broadcast) is
     accumulated in PSUM via fp16 matmuls with the W1 chunks as stationary
     operands (the a-term enters via a one-hot moving operand against the
     on-device-computed ma = mentions@W1a).
  4. One ScalarEngine activation applies  leaky_relu(hT + b1)  straight out
     of PSUM into SBUF (fp16).
  5. Layer 2 uses hrelu slices as the stationary operand: out[pair, 1] =
     hrelu_slice^T @ W_out; + b_out (activation bias) + rough (DVE add).

Pair order is "antecedent-major" (p' = j*64 + i) so the a-broadcast is a
clean 64-wide repeat; all permutation/layout work is host-side.
"""

import numpy as np

N_MENTIONS = 10000
BATCH = 512
N_ANTS = 50
EMB = 1024
PW = 64
HID = 128
N_CORES = 8
R = BATCH // N_CORES            # 64 rows per core
NPAIR = R * N_ANTS              # 3200 pairs per core
BLK = 640                       # pairs per pipeline block
NB = NPAIR // BLK               # 5 blocks
NG = BLK // 128                 # 5 layer-2 column groups per block
NCH = EMB // 128                # 8 embedding chunks
EPS = 1e-7
LEAKY = 0.01

_CACHE = {}


def _build():
    """Build + compile the (SPMD, per-core identical) Bass program."""
    if "nc" in _CACHE:
        return _CACHE["nc"]
    from concourse import bacc, mybir
    import concourse.tile as tile

    f16, f32, i16 = mybir.dt.float16, mybir.dt.float32, mybir.dt.int16
    AF = mybir.ActivationFunctionType
    nc = bacc.Bacc(num_swdge_queues=4)

    def inp(name, shape, dtype):
        return nc.declare_dram_parameter(name, list(shape), dtype, isOutput=False)

    table = inp("table", [N_MENTIONS, EMB], f16)
    idx = inp("idx", [128, NPAIR // 16], i16)
    wb = inp("wb", [128, NCH * HID], f16)
    ws = inp("ws", [128, NCH * HID], f16)
    wa = inp("wa", [128, NCH * HID], f16)
    mT = inp("mT", [128, NCH * R], f16)
    aT = inp("aT", [128, NCH * BLK], f16)
    w1p = inp("w1p", [PW, HID], f16)
    pwT = inp("pwT", [PW, NPAIR], f16)
    e64 = inp("e64", [R, NPAIR], f16)
    wout = inp("wout", [HID, 1], f16)
    b1c = inp("b1c", [HID, 1], f32)
    boutc = inp("boutc", [128, 1], f32)
    rough = inp("rough", [128, NPAIR // 128], f32)
    out = nc.declare_dram_parameter("out", [128, NPAIR // 128], f32, isOutput=True)

    with tile.TileContext(nc) as tc:
        with (
            tc.tile_pool(name="const", bufs=1) as cp,
            tc.tile_pool(name="bt", bufs=3) as btp,
            tc.tile_pool(name="st", bufs=3) as stp,
            tc.tile_pool(name="hr", bufs=2) as hrp,
            tc.tile_pool(name="sm", bufs=2) as smp,
            tc.tile_pool(name="psH", bufs=2, space="PSUM") as psH,
            tc.tile_pool(name="psS", bufs=2, space="PSUM") as psS,
            tc.tile_pool(name="psM", bufs=1, space="PSUM") as psM,
        ):
            def load(param, shape, dtype, tag):
                t = cp.tile(shape, dtype, tag=tag)
                nc.sync.dma_start(out=t[:], in_=param[:])
                return t

            idx_sb = load(idx, [128, NPAIR // 16], i16, "idx")

            # Kick off all gathers as early as possible (desc-gen runs on a
            # Q7 core pair selected by queue_num, so spreading queues lets
            # up to 4 descriptor generations run concurrently).
            bts = []
            for b in range(NB):
                bt = btp.tile([128, NCH * BLK], f16, tag="bt")
                nc.gpsimd.dma_gather(
                    out_ap=bt[:].rearrange("p (c n) -> p c n", c=NCH),
                    in_ap=table[:],
                    idxs_ap=idx_sb[:, b * (BLK // 16):(b + 1) * (BLK // 16)],
                    num_idxs=BLK,
                    num_idxs_reg=BLK,
                    elem_size=EMB,
                    transpose=True,
                    queue_num=b % 4,
                )
                bts.append(bt)

            wb_sb = load(wb, [128, NCH * HID], f16, "wb")
            ws_sb = load(ws, [128, NCH * HID], f16, "ws")
            wa_sb = load(wa, [128, NCH * HID], f16, "wa")
            mT_sb = load(mT, [128, NCH * R], f16, "mT")
            aT_sb = load(aT, [128, NCH * BLK], f16, "aT")
            w1p_sb = load(w1p, [PW, HID], f16, "w1p")
            pwT_sb = load(pwT, [PW, NPAIR], f16, "pwT")
            e64_sb = load(e64, [R, NPAIR], f16, "e64")
            wout_sb = load(wout, [HID, 1], f16, "wout")
            b1_sb = load(b1c, [HID, 1], f32, "b1c")
            bout_sb = load(boutc, [128, 1], f32, "boutc")
            rough_sb = load(rough, [128, NPAIR // 128], f32, "rough")

            scores_sb = cp.tile([128, NPAIR // 128], f32, tag="scores")

            # ma = mentions_shard @ W1a  -> [R, HID]
            ma_ps = psM.tile([R, HID], f32)
            for c in range(NCH):
                nc.tensor.matmul(
                    ma_ps[:],
                    lhsT=mT_sb[:, c * R:(c + 1) * R],
                    rhs=wa_sb[:, c * HID:(c + 1) * HID],
                    start=(c == 0),
                    stop=(c == NCH - 1),
                )
            ma_sb = cp.tile([R, HID], f16)
            nc.scalar.activation(ma_sb[:], ma_ps[:], AF.Copy)

            nsub = [(0, 512), (512, BLK)]
            for b in range(NB):
                bt = bts[b]
                st = stp.tile([128, NCH * BLK], f16, tag="st")
                nc.vector.tensor_mul(st[:], bt[:], aT_sb[:])

                hT = psH.tile([128, BLK], f32)
                for lo, hi in nsub:
                    for c in range(NCH):
                        nc.tensor.matmul(
                            hT[:, lo:hi],
                            lhsT=wb_sb[:, c * HID:(c + 1) * HID],
                            rhs=bt[:, c * BLK + lo:c * BLK + hi],
                            start=(c == 0),
                            stop=False,
                        )
                    for c in range(NCH):
                        nc.tensor.matmul(
                            hT[:, lo:hi],
                            lhsT=ws_sb[:, c * HID:(c + 1) * HID],
                            rhs=st[:, c * BLK + lo:c * BLK + hi],
                            start=False,
                            stop=False,
                        )
                    nc.tensor.matmul(
                        hT[:, lo:hi],
                        lhsT=w1p_sb[:],
                        rhs=pwT_sb[:, b * BLK + lo:b * BLK + hi],
                        start=False,
                        stop=False,
                    )
                    nc.tensor.matmul(
                        hT[:, lo:hi],
                        lhsT=ma_sb[:],
                        rhs=e64_sb[:, b * BLK + lo:b * BLK + hi],
                        start=False,
                        stop=True,
                    )

                hr = hrp.tile([128, BLK], f16)
                nc.scalar.activation(
                    hr[:], hT[:], AF.Lrelu, bias=b1_sb[:], scale=1.0, alpha=LEAKY
                )

                sc = psS.tile([128, NG], f32)
                for g in range(NG):
                    nc.tensor.matmul(
                        sc[:, g:g + 1],
                        lhsT=hr[:, g * 128:(g + 1) * 128],
                        rhs=wout_sb[:],
                        start=True,
                        stop=True,
                    )
                tmp = smp.tile([128, NG], f32)
                nc.scalar.activation(
                    tmp[:], sc[:], AF.Identity, bias=bout_sb[:], scale=1.0
                )
                nc.vector.tensor_add(
                    scores_sb[:, b * NG:(b + 1) * NG],
                    tmp[:],
                    rough_sb[:, b * NG:(b + 1) * NG],
                )

            nc.sync.dma_start(out=out[:], in_=scores_sb[:])

    nc.compile()
    _CACHE["nc"] = nc
    return nc


def _chunkT(w):
    # [1024, 128] -> [128, 8*128] fp16: column c*128+h holds W[c*128+r, h]
    return np.ascontiguousarray(
        w.reshape(NCH, 128, HID).transpose(1, 0, 2).reshape(128, NCH * HID)
    ).astype(np.float16)


def _host_shared(inputs):
    table = np.asarray(inputs["all_mentions"], np.float32).astype(np.float16)
    W1 = np.asarray(inputs["W1"], np.float32)
    w1a, w1b, w1s, w1p = W1[:1024], W1[1024:2048], W1[2048:3072], W1[3072:]
    shared = {
        "table": np.ascontiguousarray(table),
        "wb": _chunkT(w1b),
        "ws": _chunkT(w1s),
        "wa": _chunkT(w1a),
        "w1p": np.ascontiguousarray(w1p).astype(np.float16),
        "e64": np.ascontiguousarray(np.tile(np.eye(R, dtype=np.float16), (1, N_ANTS))),
        "wout": np.asarray(inputs["W_out"], np.float32).astype(np.float16),
        "b1c": np.asarray(inputs["b1"], np.float32).reshape(HID, 1).copy(),
        "boutc": np.full((128, 1), np.asarray(inputs["b_out"], np.float32).reshape(())),
    }
    return shared


def _host_core(inputs, c):
    sl = slice(c * R, (c + 1) * R)
    m = np.asarray(inputs["mentions_batch"], np.float32)[sl]          # [64, 1024]
    pw = np.asarray(inputs["pw_batch"], np.float32)[sl]               # [64, 50, 64]
    idx = np.asarray(inputs["top_indices_batch"])[sl].astype(np.int64)
    rough = np.asarray(inputs["top_rough_scores_batch"], np.float32)[sl]

    idx_perm = idx.T.reshape(NPAIR).astype(np.int16)                  # p' = j*R + i
    idx16 = np.concatenate(
        [
            np.tile(
                idx_perm[b * BLK:(b + 1) * BLK].reshape(BLK // 16, 16).T, (8, 1)
            )
            for b in range(NB)
        ],
        axis=1,
    )                                                                 # [128, 200]

    mT = m.reshape(R, NCH, 128).transpose(2, 1, 0).reshape(128, NCH * R)
    aT = np.broadcast_to(
        mT.reshape(128, NCH, 1, R), (128, NCH, BLK // R, R)
    ).reshape(128, NCH * BLK)
    pwT = pw.transpose(1, 0, 2).reshape(NPAIR, PW).T                  # [64, 3200]
    rough_pp = rough.T.reshape(NPAIR).reshape(NPAIR // 128, 128).T    # [128, 25]

    return {
        "idx": np.ascontiguousarray(idx16),
        "mT": np.ascontiguousarray(mT).astype(np.float16),
        "aT": np.ascontiguousarray(aT).astype(np.float16),
        "pwT": np.ascontiguousarray(pwT).astype(np.float16),
        "rough": np.ascontiguousarray(rough_pp).astype(np.float32),
    }


def make_in_maps(inputs):
    shared = _host_shared(inputs)
    return [{**shared, **_host_core(inputs, c)} for c in range(N_CORES)]


def assemble_output(inputs, results):
    """results: list of per-core dicts with 'out' [128, 25] -> [512, 51] f32."""
    rough = np.asarray(inputs["top_rough_scores_batch"], np.float32)
    scores = np.empty((BATCH, N_ANTS), np.float32)
    for c in range(N_CORES):
        out_flat = np.asarray(results[c]["out"], np.float32).T.reshape(NPAIR)
        scores[c * R:(c + 1) * R] = out_flat.reshape(N_ANTS, R).T
    del rough
    dummy = np.full((BATCH, 1), EPS, np.float32)
    return np.concatenate([dummy, scores], axis=1)


def run(inputs, trace=False, **kwargs):
    """Compile (cached), run on 8 cores, return (output, BassKernelResults)."""
    from concourse.bass_utils import run_bass_kernel_spmd

    nc = _build()
    in_maps = make_in_maps(inputs)
    res = run_bass_kernel_spmd(
        nc, in_maps, core_ids=list(range(N_CORES)), trace=trace, **kwargs
    )
    return assemble_output(inputs, res.results), res


def kernel(**inputs) -> np.ndarray:
    out, _ = run(inputs, trace=False)
    return out


# revision 8
# speedup vs baseline: 1.2447x; 1.0677x over previous
"""Trainium2 Bass kernel: AnaphoricityScorer (wl-coref pair FFNN scorer).

Data-parallel over the 512-row mention batch across 8 NeuronCores (64 rows
per core).  Per core (3200 pairs):

  1. The gather b = all_mentions[top_indices] is done with the GPSIMD
     dma_gather(transpose=True) custom DMA, which lands the gathered fp16
     embeddings TRANSPOSED in SBUF: out[p, c, n] = table[idx_n, 128c+p].
     That puts the contraction dim (embedding) on partitions, which is
     exactly the layout the TensorEngine needs for the moving operand.
  2. s = a * b (similarity) is one DVE multiply against a pre-broadcast
     a^T tile (host-side replication of the mention embeddings).
  3. hT[h, pair] = W1b^T b + W1s^T s + W1p^T pw + (a@W1a broadcast) is
     accumulated in PSUM via fp16 matmuls with the W1 chunks as stationary
     operands (the a-term enters via a one-hot moving operand against the
     on-device-computed ma = mentions@W1a).
  4. One ScalarEngine activation applies  leaky_relu(hT + b1)  straight out
     of PSUM into SBUF (fp16).
  5. Layer 2 uses hrelu slices as the stationary operand: out[pair, 1] =
     hrelu_slice^T @ W_out; + b_out (activation bias) + rough (DVE add).

Pair order is "antecedent-major" (p' = j*64 + i) so the a-broadcast is a
clean 64-wide repeat; all permutation/layout work is host-side.
"""

import numpy as np

N_MENTIONS = 10000
BATCH = 512
N_ANTS = 50
EMB = 1024
PW = 64
HID = 128
N_CORES = 8
R = BATCH // N_CORES            # 64 rows per core
NPAIR = R * N_ANTS              # 3200 pairs per core
BLK = 640                       # pairs per pipeline block
NB = NPAIR // BLK               # 5 blocks
NG = BLK // 128                 # 5 layer-2 column groups per block
NCH = EMB // 128                # 8 embedding chunks
EPS = 1e-7
LEAKY = 0.01

_CACHE = {}


def _build():
    """Build + compile the (SPMD, per-core identical) Bass program."""
    if "nc" in _CACHE:
        return _CACHE["nc"]
    from concourse import bacc, mybir
    import concourse.tile as tile

    f16, f32, i16 = mybir.dt.float16, mybir.dt.float32, mybir.dt.int16
    AF = mybir.ActivationFunctionType
    nc = bacc.Bacc(num_swdge_queues=4)

    def inp(name, shape, dtype):
        return nc.declare_dram_parameter(name, list(shape), dtype, isOutput=False)

    table = inp("table", [N_MENTIONS, EMB], f16)
    idx = inp("idx", [128, NPAIR // 16], i16)
    wb = inp("wb", [128, NCH * HID], f16)
    ws = inp("ws", [128, NCH * HID], f16)
    wa = inp("wa", [128, NCH * HID], f16)
    mT = inp("mT", [128, NCH * R], f16)
    aT = inp("aT", [128, NCH * BLK], f16)
    w1p = inp("w1p", [PW, HID], f16)
    pwT = inp("pwT", [PW, NPAIR], f16)
    e64 = inp("e64", [R, NPAIR], f16)
    wout = inp("wout", [HID, 1], f16)
    b1c = inp("b1c", [HID, 1], f32)
    boutc = inp("boutc", [128, 1], f32)
    rough = inp("rough", [128, NPAIR // 128], f32)
    out = nc.declare_dram_parameter("out", [128, NPAIR // 128], f32, isOutput=True)

    with tile.TileContext(nc) as tc:
        with (
            tc.tile_pool(name="const", bufs=1) as cp,
            tc.tile_pool(name="bt", bufs=5) as btp,
            tc.tile_pool(name="st", bufs=3) as stp,
            tc.tile_pool(name="hr", bufs=2) as hrp,
            tc.tile_pool(name="sm", bufs=2) as smp,
            tc.tile_pool(name="psH", bufs=2, space="PSUM") as psH,
            tc.tile_pool(name="psS", bufs=2, space="PSUM") as psS,
            tc.tile_pool(name="psM", bufs=1, space="PSUM") as psM,
        ):
            def load(param, shape, dtype, tag):
                t = cp.tile(shape, dtype, tag=tag)
                nc.sync.dma_start(out=t[:], in_=param[:])
                return t

            # idx goes over the Pool engine's own SWDGE path so it lands
            # before the (long) sync-queue input loads and unblocks the
            # first gather's descriptor generation immediately.
            idx_sb = cp.tile([128, NPAIR // 16], i16, tag="idx")
            nc.gpsimd.dma_start(out=idx_sb[:], in_=idx[:])

            # Kick off all gathers as early as possible (desc-gen runs on a
            # Q7 core pair selected by queue_num, so spreading queues lets
            # up to 4 descriptor generations run concurrently).
            bts = []
            for b in range(NB):
                bt = btp.tile([128, NCH * BLK], f16, tag="bt")
                nc.gpsimd.dma_gather(
                    out_ap=bt[:].rearrange("p (c n) -> p c n", c=NCH),
                    in_ap=table[:],
                    idxs_ap=idx_sb[:, b * (BLK // 16):(b + 1) * (BLK // 16)],
                    num_idxs=BLK,
                    num_idxs_reg=BLK,
                    elem_size=EMB,
                    transpose=True,
                    queue_num=b % 4,
                )
                bts.append(bt)

            wb_sb = load(wb, [128, NCH * HID], f16, "wb")
            ws_sb = load(ws, [128, NCH * HID], f16, "ws")
            wa_sb = load(wa, [128, NCH * HID], f16, "wa")
            mT_sb = load(mT, [128, NCH * R], f16, "mT")
            aT_sb = load(aT, [128, NCH * BLK], f16, "aT")
            w1p_sb = load(w1p, [PW, HID], f16, "w1p")
            pwT_sb = load(pwT, [PW, NPAIR], f16, "pwT")
            e64_sb = load(e64, [R, NPAIR], f16, "e64")
            wout_sb = load(wout, [HID, 1], f16, "wout")
            b1_sb = load(b1c, [HID, 1], f32, "b1c")
            bout_sb = load(boutc, [128, 1], f32, "boutc")
            rough_sb = load(rough, [128, NPAIR // 128], f32, "rough")

            scores_sb = cp.tile([128, NPAIR // 128], f32, tag="scores")

            # ma = mentions_shard @ W1a  -> [R, HID]
            ma_ps = psM.tile([R, HID], f32)
            for c in range(NCH):
                nc.tensor.matmul(
                    ma_ps[:],
                    lhsT=mT_sb[:, c * R:(c + 1) * R],
                    rhs=wa_sb[:, c * HID:(c + 1) * HID],
                    start=(c == 0),
                    stop=(c == NCH - 1),
                )
            ma_sb = cp.tile([R, HID], f16)
            nc.scalar.activation(ma_sb[:], ma_ps[:], AF.Copy)

            nsub = [(0, 512), (512, BLK)]
            for b in range(NB):
                bt = bts[b]
                st = stp.tile([128, NCH * BLK], f16, tag="st")
                nc.vector.tensor_mul(st[:], bt[:], aT_sb[:])

                hT = psH.tile([128, BLK], f32)
                for lo, hi in nsub:
                    for c in range(NCH):
                        nc.tensor.matmul(
                            hT[:, lo:hi],
                            lhsT=wb_sb[:, c * HID:(c + 1) * HID],
                            rhs=bt[:, c * BLK + lo:c * BLK + hi],
                            start=(c == 0),
                            stop=False,
                        )
                    for c in range(NCH):
                        nc.tensor.matmul(
                            hT[:, lo:hi],
                            lhsT=ws_sb[:, c * HID:(c + 1) * HID],
                            rhs=st[:, c * BLK + lo:c * BLK + hi],
                            start=False,
                            stop=False,
                        )
                    nc.tensor.matmul(
                        hT[:, lo:hi],
                        lhsT=w1p_sb[:],
                        rhs=pwT_sb[:, b * BLK + lo:b * BLK + hi],
                        start=False,
                        stop=False,
                    )
                    nc.tensor.matmul(
                        hT[:, lo:hi],
                        lhsT=ma_sb[:],
                        rhs=e64_sb[:, b * BLK + lo:b * BLK + hi],
                        start=False,
                        stop=True,
                    )

                hr = hrp.tile([128, BLK], f16)
                nc.scalar.activation(
                    hr[:], hT[:], AF.Lrelu, bias=b1_sb[:], scale=1.0, alpha=LEAKY
                )

                sc = psS.tile([128, NG], f32)
                for g in range(NG):
                    nc.tensor.matmul(
                        sc[:, g:g + 1],
                        lhsT=hr[:, g * 128:(g + 1) * 128],
                        rhs=wout_sb[:],
                        start=True,
                        stop=True,
                    )
                tmp = smp.tile([128, NG], f32)
                nc.scalar.activation(
                    tmp[:], sc[:], AF.Identity, bias=bout_sb[:], scale=1.0
                )
                nc.vector.tensor_add(
                    scores_sb[:, b * NG:(b + 1) * NG],
                    tmp[:],
                    rough_sb[:, b * NG:(b + 1) * NG],
                )

            nc.sync.dma_start(out=out[:], in_=scores_sb[:])

    nc.compile()
    _CACHE["nc"] = nc
    return nc


def _chunkT(w):
    # [1024, 128] -> [128, 8*128] fp16: column c*128+h holds W[c*128+r, h]
    return np.ascontiguousarray(
        w.reshape(NCH, 128, HID).transpose(1, 0, 2).reshape(128, NCH * HID)
    ).astype(np.float16)


def _host_shared(inputs):
    table = np.asarray(inputs["all_mentions"], np.float32).astype(np.float16)
    W1 = np.asarray(inputs["W1"], np.float32)
    w1a, w1b, w1s, w1p = W1[:1024], W1[1024:2048], W1[2048:3072], W1[3072:]
    shared = {
        "table": np.ascontiguousarray(table),
        "wb": _chunkT(w1b),
        "ws": _chunkT(w1s),
        "wa": _chunkT(w1a),
        "w1p": np.ascontiguousarray(w1p).astype(np.float16),
        "e64": np.ascontiguousarray(np.tile(np.eye(R, dtype=np.float16), (1, N_ANTS))),
        "wout": np.asarray(inputs["W_out"], np.float32).astype(np.float16),
        "b1c": np.asarray(inputs["b1"], np.float32).reshape(HID, 1).copy(),
        "boutc": np.full((128, 1), np.asarray(inputs["b_out"], np.float32).reshape(())),
    }
    return shared


def _host_core(inputs, c):
    sl = slice(c * R, (c + 1) * R)
    m = np.asarray(inputs["mentions_batch"], np.float32)[sl]          # [64, 1024]
    pw = np.asarray(inputs["pw_batch"], np.float32)[sl]               # [64, 50, 64]
    idx = np.asarray(inputs["top_indices_batch"])[sl].astype(np.int64)
    rough = np.asarray(inputs["top_rough_scores_batch"], np.float32)[sl]

    idx_perm = idx.T.reshape(NPAIR).astype(np.int16)                  # p' = j*R + i
    idx16 = np.concatenate(
        [
            np.tile(
                idx_perm[b * BLK:(b + 1) * BLK].reshape(BLK // 16, 16).T, (8, 1)
            )
            for b in range(NB)
        ],
        axis=1,
    )                                                                 # [128, 200]

    mT = m.reshape(R, NCH, 128).transpose(2, 1, 0).reshape(128, NCH * R)
    aT = np.broadcast_to(
        mT.reshape(128, NCH, 1, R), (128, NCH, BLK // R, R)
    ).reshape(128, NCH * BLK)
    pwT = pw.transpose(1, 0, 2).reshape(NPAIR, PW).T                  # [64, 3200]
    rough_pp = rough.T.reshape(NPAIR).reshape(NPAIR // 128, 128).T    # [128, 25]

    return {
        "idx": np.ascontiguousarray(idx16),
        "mT": np.ascontiguousarray(mT).astype(np.float16),
        "aT": np.ascontiguousarray(aT).astype(np.float16),
        "pwT": np.ascontiguousarray(pwT).astype(np.float16),
        "rough": np.ascontiguousarray(rough_pp).astype(np.float32),
    }


def make_in_maps(inputs):
    shared = _host_shared(inputs)
    return [{**shared, **_host_core(inputs, c)} for c in range(N_CORES)]


def assemble_output(inputs, results):
    """results: list of per-core dicts with 'out' [128, 25] -> [512, 51] f32."""
    rough = np.asarray(inputs["top_rough_scores_batch"], np.float32)
    scores = np.empty((BATCH, N_ANTS), np.float32)
    for c in range(N_CORES):
        out_flat = np.asarray(results[c]["out"], np.float32).T.reshape(NPAIR)
        scores[c * R:(c + 1) * R] = out_flat.reshape(N_ANTS, R).T
    del rough
    dummy = np.full((BATCH, 1), EPS, np.float32)
    return np.concatenate([dummy, scores], axis=1)


def run(inputs, trace=False, **kwargs):
    """Compile (cached), run on 8 cores, return (output, BassKernelResults)."""
    from concourse.bass_utils import run_bass_kernel_spmd

    nc = _build()
    in_maps = make_in_maps(inputs)
    res = run_bass_kernel_spmd(
        nc, in_maps, core_ids=list(range(N_CORES)), trace=trace, **kwargs
    )
    return assemble_output(inputs, res.results), res


def kernel(**inputs) -> np.ndarray:
    out, _ = run(inputs, trace=False)
    return out


# revision 13
# speedup vs baseline: 1.3069x; 1.0500x over previous
"""Trainium2 Bass kernel: AnaphoricityScorer (wl-coref pair FFNN scorer).

Data-parallel over the 512-row mention batch across 8 NeuronCores (64 rows
per core).  Per core (3200 pairs):

  1. The gather b = all_mentions[top_indices] is done with the GPSIMD
     dma_gather(transpose=True) custom DMA, which lands the gathered fp16
     embeddings TRANSPOSED in SBUF: out[p, c, n] = table[idx_n, 128c+p].
     That puts the contraction dim (embedding) on partitions, which is
     exactly the layout the TensorEngine needs for the moving operand.
  2. s = a * b (similarity) is one DVE multiply against a pre-broadcast
     a^T tile (host-side replication of the mention embeddings).
  3. hT[h, pair] = W1b^T b + W1s^T s + W1p^T pw + (a@W1a broadcast) is
     accumulated in PSUM via fp16 matmuls with the W1 chunks as stationary
     operands (the a-term enters via a one-hot moving operand against the
     on-device-computed ma = mentions@W1a).
  4. One ScalarEngine activation applies  leaky_relu(hT + b1)  straight out
     of PSUM into SBUF (fp16).
  5. Layer 2 uses hrelu slices as the stationary operand: out[pair, 1] =
     hrelu_slice^T @ W_out; + b_out (activation bias) + rough (DVE add).

Pair order is "antecedent-major" (p' = j*64 + i) so the a-broadcast is a
clean 64-wide repeat; all permutation/layout work is host-side.
"""

import numpy as np

N_MENTIONS = 10000
BATCH = 512
N_ANTS = 50
EMB = 1024
PW = 64
HID = 128
N_CORES = 8
R = BATCH // N_CORES            # 64 rows per core
NPAIR = R * N_ANTS              # 3200 pairs per core
BLK = 640                       # pairs per pipeline block
NB = NPAIR // BLK               # 5 blocks
NG = BLK // 128                 # 5 layer-2 column groups per block
NCH = EMB // 128                # 8 embedding chunks
EPS = 1e-7
LEAKY = 0.01

_CACHE = {}


def _build():
    """Build + compile the (SPMD, per-core identical) Bass program."""
    if "nc" in _CACHE:
        return _CACHE["nc"]
    from concourse import bacc, mybir
    import concourse.tile as tile

    f16, f32, i16 = mybir.dt.float16, mybir.dt.float32, mybir.dt.int16
    AF = mybir.ActivationFunctionType
    nc = bacc.Bacc(num_swdge_queues=4)

    def inp(name, shape, dtype):
        return nc.declare_dram_parameter(name, list(shape), dtype, isOutput=False)

    table = inp("table", [N_MENTIONS, EMB], f16)
    idx = inp("idx", [128, NPAIR // 16], i16)
    wb = inp("wb", [128, NCH * HID], f16)
    ws = inp("ws", [128, NCH * HID], f16)
    wa = inp("wa", [128, NCH * HID], f16)
    mT = inp("mT", [128, NCH * R], f16)
    w1p = inp("w1p", [PW, HID], f16)
    pwT = inp("pwT", [PW, NPAIR], f16)
    e64 = inp("e64", [R, NPAIR], f16)
    wout = inp("wout", [HID, 1], f16)
    b1c = inp("b1c", [HID, 1], f32)
    boutc = inp("boutc", [128, 1], f32)
    rough = inp("rough", [128, NPAIR // 128], f32)
    out = nc.declare_dram_parameter("out", [128, NPAIR // 128], f32, isOutput=True)

    with tile.TileContext(nc) as tc:
        with (
            tc.tile_pool(name="const", bufs=1) as cp,
            tc.tile_pool(name="bt", bufs=5) as btp,
            tc.tile_pool(name="st", bufs=3) as stp,
            tc.tile_pool(name="hr", bufs=2) as hrp,
            tc.tile_pool(name="sm", bufs=2) as smp,
            tc.tile_pool(name="psH", bufs=2, space="PSUM") as psH,
            tc.tile_pool(name="psS", bufs=2, space="PSUM") as psS,
            tc.tile_pool(name="psM", bufs=1, space="PSUM") as psM,
        ):
            def load(param, shape, dtype, tag):
                # Constants ride the Scalar HWDGE ring so the idx load (the
                # critical input, alone on the Sync ring) completes at once.
                t = cp.tile(shape, dtype, tag=tag)
                nc.scalar.dma_start(out=t[:], in_=param[:])
                return t

            idx_sb = cp.tile([128, NPAIR // 16], i16, tag="idx")
            nc.sync.dma_start(out=idx_sb[:], in_=idx[:])

            # Kick off all gathers as early as possible (desc-gen runs on a
            # Q7 core pair selected by queue_num, so spreading queues lets
            # up to 4 descriptor generations run concurrently).
            bts = []
            for b in range(NB):
                bt = btp.tile([128, NCH * BLK], f16, tag="bt")
                nc.gpsimd.dma_gather(
                    out_ap=bt[:].rearrange("p (c n) -> p c n", c=NCH),
                    in_ap=table[:],
                    idxs_ap=idx_sb[:, b * (BLK // 16):(b + 1) * (BLK // 16)],
                    num_idxs=BLK,
                    num_idxs_reg=BLK,
                    elem_size=EMB,
                    transpose=True,
                    queue_num=b % 4,
                )
                bts.append(bt)

            wb_sb = load(wb, [128, NCH * HID], f16, "wb")
            ws_sb = load(ws, [128, NCH * HID], f16, "ws")
            wa_sb = load(wa, [128, NCH * HID], f16, "wa")
            mT_sb = load(mT, [128, NCH * R], f16, "mT")
            # aT = per-block a^T broadcast (j-repeat of mT) built on-device.
            aT_sb = cp.tile([128, NCH * BLK], f16, tag="aT")
            nc.vector.tensor_copy(
                aT_sb[:].rearrange("p (c j i) -> p c j i", c=NCH, j=BLK // R),
                mT_sb[:].rearrange("p (c i) -> p c i", c=NCH)[:, :, None, :]
                .broadcast_to([128, NCH, BLK // R, R]),
            )
            w1p_sb = load(w1p, [PW, HID], f16, "w1p")
            pwT_sb = load(pwT, [PW, NPAIR], f16, "pwT")
            e64_sb = load(e64, [R, NPAIR], f16, "e64")
            wout_sb = load(wout, [HID, 1], f16, "wout")
            b1_sb = load(b1c, [HID, 1], f32, "b1c")
            bout_sb = load(boutc, [128, 1], f32, "boutc")
            rough_sb = load(rough, [128, NPAIR // 128], f32, "rough")

            scores_sb = cp.tile([128, NPAIR // 128], f32, tag="scores")

            # ma = mentions_shard @ W1a  -> [R, HID]
            ma_ps = psM.tile([R, HID], f32)
            for c in range(NCH):
                nc.tensor.matmul(
                    ma_ps[:],
                    lhsT=mT_sb[:, c * R:(c + 1) * R],
                    rhs=wa_sb[:, c * HID:(c + 1) * HID],
                    start=(c == 0),
                    stop=(c == NCH - 1),
                )
            ma_sb = cp.tile([R, HID], f16)
            nc.scalar.activation(ma_sb[:], ma_ps[:], AF.Copy)

            nsub = [(0, 512), (512, BLK)]
            for b in range(NB):
                bt = bts[b]
                st = stp.tile([128, NCH * BLK], f16, tag="st")
                nc.vector.tensor_mul(st[:], bt[:], aT_sb[:])

                hT = psH.tile([128, BLK], f32)
                for lo, hi in nsub:
                    for c in range(NCH):
                        nc.tensor.matmul(
                            hT[:, lo:hi],
                            lhsT=wb_sb[:, c * HID:(c + 1) * HID],
                            rhs=bt[:, c * BLK + lo:c * BLK + hi],
                            start=(c == 0),
                            stop=False,
                        )
                    for c in range(NCH):
                        nc.tensor.matmul(
                            hT[:, lo:hi],
                            lhsT=ws_sb[:, c * HID:(c + 1) * HID],
                            rhs=st[:, c * BLK + lo:c * BLK + hi],
                            start=False,
                            stop=False,
                        )
                    nc.tensor.matmul(
                        hT[:, lo:hi],
                        lhsT=w1p_sb[:],
                        rhs=pwT_sb[:, b * BLK + lo:b * BLK + hi],
                        start=False,
                        stop=False,
                    )
                    nc.tensor.matmul(
                        hT[:, lo:hi],
                        lhsT=ma_sb[:],
                        rhs=e64_sb[:, b * BLK + lo:b * BLK + hi],
                        start=False,
                        stop=True,
                    )

                hr = hrp.tile([128, BLK], f16)
                nc.scalar.activation(
                    hr[:], hT[:], AF.Lrelu, bias=b1_sb[:], scale=1.0, alpha=LEAKY
                )

                sc = psS.tile([128, NG], f32)
                for g in range(NG):
                    nc.tensor.matmul(
                        sc[:, g:g + 1],
                        lhsT=hr[:, g * 128:(g + 1) * 128],
                        rhs=wout_sb[:],
                        start=True,
                        stop=True,
                    )
                tmp = smp.tile([128, NG], f32)
                nc.scalar.activation(
                    tmp[:], sc[:], AF.Identity, bias=bout_sb[:], scale=1.0
                )
                nc.vector.tensor_add(
                    scores_sb[:, b * NG:(b + 1) * NG],
                    tmp[:],
                    rough_sb[:, b * NG:(b + 1) * NG],
                )

            nc.sync.dma_start(out=out[:], in_=scores_sb[:])

    nc.compile()
    _CACHE["nc"] = nc
    return nc


def _chunkT(w):
    # [1024, 128] -> [128, 8*128] fp16: column c*128+h holds W[c*128+r, h]
    return np.ascontiguousarray(
        w.reshape(NCH, 128, HID).transpose(1, 0, 2).reshape(128, NCH * HID)
    ).astype(np.float16)


def _host_shared(inputs):
    table = np.asarray(inputs["all_mentions"], np.float32).astype(np.float16)
    W1 = np.asarray(inputs["W1"], np.float32)
    w1a, w1b, w1s, w1p = W1[:1024], W1[1024:2048], W1[2048:3072], W1[3072:]
    shared = {
        "table": np.ascontiguousarray(table),
        "wb": _chunkT(w1b),
        "ws": _chunkT(w1s),
        "wa": _chunkT(w1a),
        "w1p": np.ascontiguousarray(w1p).astype(np.float16),
        "e64": np.ascontiguousarray(np.tile(np.eye(R, dtype=np.float16), (1, N_ANTS))),
        "wout": np.asarray(inputs["W_out"], np.float32).astype(np.float16),
        "b1c": np.asarray(inputs["b1"], np.float32).reshape(HID, 1).copy(),
        "boutc": np.full((128, 1), np.asarray(inputs["b_out"], np.float32).reshape(())),
    }
    return shared


def _host_core(inputs, c):
    sl = slice(c * R, (c + 1) * R)
    m = np.asarray(inputs["mentions_batch"], np.float32)[sl]          # [64, 1024]
    pw = np.asarray(inputs["pw_batch"], np.float32)[sl]               # [64, 50, 64]
    idx = np.asarray(inputs["top_indices_batch"])[sl].astype(np.int64)
    rough = np.asarray(inputs["top_rough_scores_batch"], np.float32)[sl]

    idx_perm = idx.T.reshape(NPAIR).astype(np.int16)                  # p' = j*R + i
    idx16 = np.concatenate(
        [
            np.tile(
                idx_perm[b * BLK:(b + 1) * BLK].reshape(BLK // 16, 16).T, (8, 1)
            )
            for b in range(NB)
        ],
        axis=1,
    )                                                                 # [128, 200]

    mT = m.reshape(R, NCH, 128).transpose(2, 1, 0).reshape(128, NCH * R)
    pwT = pw.transpose(1, 0, 2).reshape(NPAIR, PW).T                  # [64, 3200]
    rough_pp = rough.T.reshape(NPAIR).reshape(NPAIR // 128, 128).T    # [128, 25]

    return {
        "idx": np.ascontiguousarray(idx16),
        "mT": np.ascontiguousarray(mT).astype(np.float16),
        "pwT": np.ascontiguousarray(pwT).astype(np.float16),
        "rough": np.ascontiguousarray(rough_pp).astype(np.float32),
    }


def make_in_maps(inputs):
    shared = _host_shared(inputs)
    return [{**shared, **_host_core(inputs, c)} for c in range(N_CORES)]


def assemble_output(inputs, results):
    """results: list of per-core dicts with 'out' [128, 25] -> [512, 51] f32."""
    rough = np.asarray(inputs["top_rough_scores_batch"], np.float32)
    scores = np.empty((BATCH, N_ANTS), np.float32)
    for c in range(N_CORES):
        out_flat = np.asarray(results[c]["out"], np.float32).T.reshape(NPAIR)
        scores[c * R:(c + 1) * R] = out_flat.reshape(N_ANTS, R).T
    del rough
    dummy = np.full((BATCH, 1), EPS, np.float32)
    return np.concatenate([dummy, scores], axis=1)


def run(inputs, trace=False, **kwargs):
    """Compile (cached), run on 8 cores, return (output, BassKernelResults)."""
    from concourse.bass_utils import run_bass_kernel_spmd

    nc = _build()
    in_maps = make_in_maps(inputs)
    res = run_bass_kernel_spmd(
        nc, in_maps, core_ids=list(range(N_CORES)), trace=trace, **kwargs
    )
    return assemble_output(inputs, res.results), res


def kernel(**inputs) -> np.ndarray:
    out, _ = run(inputs, trace=False)
    return out
